# revision 6
# baseline (speedup 1.0000x reference)
"""GAU (gated attention unit) Bass kernel for Trainium2, 8 NeuronCores — v2.

Sharding: 8 cores = 4 batches x 2 sequence halves. Each core computes the
full k/v for its batch and the attention output rows for its half.

Changes vs baseline:
  - LN rstd via fast-inverse-sqrt (bit trick + 2 Newton steps) on DVE:
    no Sqrt on Act, so the single silu_and_others table (silu/identity/
    relu/square) stays loaded forever -> 1 act-table load vs 54.
  - Projections (v/Z/gate) run in bf16 (weights shipped bf16);
    f32r transposes feed a bf16 nT via the PSUM round-trip copy.
  - Gate projected directly in transposed GT[o,i] layout (bias per
    partition, no ones-matmul; no HBM gate spill).
  - Phase C: per-jt software pipeline sim -> relu (Act) -> square (DVE)
    -> A@v accumulation in VT[o,i] layout; output projection consumes
    VT directly (no PE transposes in phase C), and each block's output
    projection is emitted inside the next block's jt loop so PE never
    drains on the DVE epilogue. The last 4 j-tiles accumulate oc-major
    so the next block's A@v never waits on the vgt epilogue chain.
    A@v psum pairs share banks: only slice 0 of each pair uses
    start=True (HW zeroes the whole bank on start).
  - Weight staging hoisted out of the rep loop.
"""
import sys

sys.path.insert(0, "/opt/trn_rl_repo")

import numpy as np

import concourse.bass as bass
import concourse.mybir as mybir
from concourse import bacc
from concourse.masks import make_identity
from concourse.tile import TileContext

F32 = mybir.dt.float32
F32R = mybir.dt.float32r
BF16 = mybir.dt.bfloat16
I32 = mybir.dt.int32
AF = mybir.ActivationFunctionType
OP = mybir.AluOpType

S = 4096          # full sequence
SH = 2048         # per-core q rows
D = 512           # model dim
HID = 1024        # v / gate width
H2 = 2048         # 2*HID
QK = 128
OUT = 8
NKV = S // 128    # 32 kv seq tiles
NQT = SH // 128   # 16 q seq tiles
NT = NKV + NQT    # 48 stats tiles
IB = 256          # phase-C i-block rows
NIB = SH // IB    # 8 blocks
NCORES = 8
START_LAG = 4     # sim -> A@v software pipeline depth

_nc_cache = None


def _build(nreps=1, debug=False):
    nc = bacc.Bacc()

    xkv = nc.dram_tensor("xkv", [S, D], F32, kind="ExternalInput")
    xq = nc.dram_tensor("xq", [SH, D], F32, kind="ExternalInput")
    # projection weights shipped bf16 (converted host-side), DMA'd straight
    # into bf16 SBUF tiles with no staging copy
    wh = nc.dram_tensor("wh", [D, H2], BF16, kind="ExternalInput")
    bh = nc.dram_tensor("bh", [1, H2], BF16, kind="ExternalInput")
    wqk = nc.dram_tensor("wqk", [D, QK], BF16, kind="ExternalInput")
    bqk = nc.dram_tensor("bqk", [QK, 1], F32, kind="ExternalInput")
    gam0 = nc.dram_tensor("gam0", [QK, 1], F32, kind="ExternalInput")
    bet0 = nc.dram_tensor("bet0", [QK, 1], F32, kind="ExternalInput")
    gam1 = nc.dram_tensor("gam1", [QK, 1], F32, kind="ExternalInput")
    bet1 = nc.dram_tensor("bet1", [QK, 1], F32, kind="ExternalInput")
    wo = nc.dram_tensor("wo", [128, 8, OUT], F32, kind="ExternalInput")
    bo = nc.dram_tensor("bo", [1, OUT], F32, kind="ExternalInput")
    bhg = nc.dram_tensor("bhg", [128, 8], F32, kind="ExternalInput")
    out_d = nc.dram_tensor("out", [SH, OUT], F32, kind="ExternalOutput")
    dbg = {}
    if debug:
        dbg["kt"] = nc.dram_tensor("dbg_kt", [128, S], F32R, kind="ExternalOutput")
        dbg["qt"] = nc.dram_tensor("dbg_qt", [128, SH], F32R, kind="ExternalOutput")
        dbg["v"] = nc.dram_tensor("dbg_v", [128, NKV * HID], BF16, kind="ExternalOutput")
        dbg["gt"] = nc.dram_tensor("dbg_gt", [128, 8 * SH], F32R, kind="ExternalOutput")
        dbg["mv"] = nc.dram_tensor("dbg_mv", [128, NT * 2], F32, kind="ExternalOutput")
        dbg["rstd2"] = nc.dram_tensor("dbg_rstd2", [128, NT], F32, kind="ExternalOutput")
        dbg["nmr"] = nc.dram_tensor("dbg_nmr", [128, NT], F32, kind="ExternalOutput")
        dbg["qt_early"] = nc.dram_tensor("dbg_qt_early", [128, SH], F32R, kind="ExternalOutput")
        dbg["gt_early"] = nc.dram_tensor("dbg_gt_early", [128, 8 * SH], F32R, kind="ExternalOutput")

    with TileContext(nc) as tc:
        with (
            tc.tile_pool(name="persist", bufs=1) as pers,
            tc.tile_pool(name="vpool", bufs=1) as vpool,
        ):
            # ---- persistent constants ----
            identity = pers.tile([128, 128], F32, tag="ident")
            make_identity(nc, identity)
            ident_r = pers.tile([128, 128], F32R, tag="identr")
            nc.vector.tensor_copy(out=ident_r, in_=identity)

            bqk_col = pers.tile([128, 1], F32, tag="bqk")
            nc.sync.dma_start(out=bqk_col, in_=bqk[:])
            gam0_c = pers.tile([128, 1], F32, tag="g0")
            nc.sync.dma_start(out=gam0_c, in_=gam0[:])
            bet0_c = pers.tile([128, 1], F32, tag="be0")
            nc.sync.dma_start(out=bet0_c, in_=bet0[:])
            gam1_c = pers.tile([128, 1], F32, tag="g1")
            nc.sync.dma_start(out=gam1_c, in_=gam1[:])
            bet1_c = pers.tile([128, 1], F32, tag="be1")
            nc.sync.dma_start(out=bet1_c, in_=bet1[:])
            wo_t = pers.tile([128, 8, OUT], F32, tag="wo")
            nc.sync.dma_start(out=wo_t, in_=wo[:])
            wo_bf = pers.tile([128, 8, OUT], BF16, tag="wobf")
            nc.vector.tensor_copy(out=wo_bf, in_=wo_t)
            bo_bc = pers.tile([128, OUT], F32, tag="bo")
            nc.sync.dma_start(out=bo_bc, in_=bo[:].to_broadcast([128, OUT]))
            bhg_t = pers.tile([128, 8], F32, tag="bhg")
            nc.sync.dma_start(out=bhg_t, in_=bhg[:])

            # ---- staged weights (outside the rep loop, direct bf16 DMA) ----
            whr = pers.tile([128, 4, H2], BF16, tag="whr")
            for c in range(4):
                nc.sync.dma_start(
                    out=whr[:, c, :], in_=wh[c * 128 : (c + 1) * 128, :]
                )
            wqkr = pers.tile([128, 4, QK], BF16, tag="wqkr")
            for c in range(4):
                nc.sync.dma_start(
                    out=wqkr[:, c, :], in_=wqk[c * 128 : (c + 1) * 128, :]
                )
            ones_row = pers.tile([1, 128], BF16, tag="ones")
            nc.vector.memset(ones_row, 1.0)
            bh_row = pers.tile([1, HID], BF16, tag="bhr")
            nc.sync.dma_start(out=bh_row, in_=bh[0:1, 0:HID])

            # ---- persistent activations ----
            v_sb = vpool.tile([128, NKV, HID], BF16, tag="v")
            kt_sb = pers.tile([128, S], F32R, tag="kt")
            qt_sb = pers.tile([128, SH], F32R, tag="qt")
            gt_sb = pers.tile([128, 8, SH], BF16, tag="gt")
            mvall = pers.tile([128, NT, 2], F32, tag="mvall")
            rstd2_all = pers.tile([128, NT], F32, tag="rstd2")
            nmr_all = pers.tile([128, NT], F32, tag="nmr")

            sbufs = dict(
                ident_r=ident_r, ones_row=ones_row,
                bqk_col=bqk_col, gam0_c=gam0_c, bet0_c=bet0_c,
                gam1_c=gam1_c, bet1_c=bet1_c, wo_bf=wo_bf, bo_bc=bo_bc,
                bhg_t=bhg_t, whr=whr, wqkr=wqkr, bh_row=bh_row,
                v_sb=v_sb, kt_sb=kt_sb, qt_sb=qt_sb, gt_sb=gt_sb,
                mvall=mvall, rstd2_all=rstd2_all, nmr_all=nmr_all,
            )
            drams = dict(xkv=xkv, xq=xq, out_d=out_d, dbg=dbg)

            import contextlib

            rep_ctx = (
                tc.For_i(0, nreps, 1) if nreps > 1 else contextlib.nullcontext()
            )
            with rep_ctx:
                _emit_body(nc, tc, drams, sbufs)

    nc.compile()
    return nc


def _emit_body(nc, tc, drams, sb):
    xkv, xq, out_d = drams["xkv"], drams["xq"], drams["out_d"]
    ident_r = sb["ident_r"]
    ones_row = sb["ones_row"]
    bqk_col = sb["bqk_col"]
    gam0_c, bet0_c = sb["gam0_c"], sb["bet0_c"]
    gam1_c, bet1_c = sb["gam1_c"], sb["bet1_c"]
    wo_bf, bo_bc, bhg_t = sb["wo_bf"], sb["bo_bc"], sb["bhg_t"]
    whr, wqkr, bh_row = sb["whr"], sb["wqkr"], sb["bh_row"]
    v_sb, kt_sb, qt_sb, gt_sb = sb["v_sb"], sb["kt_sb"], sb["qt_sb"], sb["gt_sb"]
    mvall = sb["mvall"]
    rstd2_all, nmr_all = sb["rstd2_all"], sb["nmr_all"]

    def tile_src(idx):
        if idx < NKV:
            return xkv, idx * 128
        return xq, (idx - NKV) * 128

    def group_rsqrt(rp, idx0, n=4):
        """rstd2_all[:, idx0:idx0+n] = rsqrt(var + eps) via bit trick +
        2 Newton steps; nmr_all = -mean * rstd2. All on DVE."""
        ve = rp.tile([128, n], F32, tag="ve")
        nc.vector.tensor_scalar_add(out=ve, in0=mvall[:, idx0 : idx0 + n, 1],
                                    scalar1=1e-5)
        ib1 = rp.tile([128, n], I32, tag="ib1")
        nc.vector.tensor_scalar(
            out=ib1, in0=ve.bitcast(I32), scalar1=1, scalar2=None,
            op0=OP.arith_shift_right,
        )
        ib2 = rp.tile([128, n], I32, tag="ib2")
        # C - a == -(a - C); bitwise+arith can't mix in one tensor_scalar
        nc.vector.tensor_scalar(
            out=ib2, in0=ib1, scalar1=0x5F3759DF, scalar2=-1,
            op0=OP.subtract, op1=OP.mult,
        )
        y = ib2.bitcast(F32)
        for it in range(2):
            yy = rp.tile([128, n], F32, tag=f"yy{it}")
            nc.vector.tensor_mul(out=yy, in0=y, in1=y)
            xyy = rp.tile([128, n], F32, tag=f"xyy{it}")
            nc.vector.tensor_mul(out=xyy, in0=yy, in1=ve)
            h = rp.tile([128, n], F32, tag=f"h{it}")
            nc.vector.tensor_scalar(
                out=h, in0=xyy, scalar1=-0.5, scalar2=1.5,
                op0=OP.mult, op1=OP.add,
            )
            if it == 0:
                ynext = rp.tile([128, n], F32, tag=f"y{it}")
            else:
                ynext = rstd2_all[:, idx0 : idx0 + n]
            nc.vector.tensor_mul(out=ynext, in0=y, in1=h)
            y = ynext
        nc.vector.scalar_tensor_tensor(
            out=nmr_all[:, idx0 : idx0 + n], in0=mvall[:, idx0 : idx0 + n, 0],
            scalar=-1.0, in1=rstd2_all[:, idx0 : idx0 + n],
            op0=OP.mult, op1=OP.mult,
        )

    def load_stats(pools, idx):
        """DMA one x tile and push bn stats into mvall[:, idx]."""
        xp, stp = pools
        src, row0 = tile_src(idx)
        xt = xp.tile([128, D], F32, tag="xa")
        nc.sync.dma_start(out=xt, in_=src[row0 : row0 + 128, :])
        st = stp.tile([128, 6], F32, tag="bn")
        nc.vector.bn_stats(out=st, in_=xt)
        nc.vector.bn_aggr(out=mvall[:, idx, :], in_=st)
        return xt

    def ln_transpose(npool, psTrp, xt, idx, nT, t):
        nsc = npool.tile([128, D], F32R, tag="nsc")
        # LN apply on Act: x*rstd + (-mean*rstd)
        nc.scalar.activation(
            out=nsc, in_=xt, func=AF.Identity,
            bias=nmr_all[:, idx : idx + 1], scale=rstd2_all[:, idx : idx + 1],
        )
        ptr = psTrp.tile([128, 4, 128], F32R, tag="ptr")
        for c in range(4):
            nc.tensor.transpose(
                ptr[:, c, :], nsc[:, c * 128 : (c + 1) * 128], ident_r
            )
        nc.vector.tensor_copy(out=nT[:, :, t * 128 : (t + 1) * 128], in_=ptr)

    def z_proj(psZp, zp, nT, dst, gam_c, bet_c):
        psz = psZp.tile([128, 512], F32, tag="psz")
        for c in range(4):
            nc.tensor.matmul(
                psz, wqkr[:, c, :], nT[:, c, :], start=(c == 0), stop=(c == 3)
            )
        zs = zp.tile([128, 512], F32, tag="zs")
        nc.scalar.activation(out=zs, in_=psz, func=AF.Silu, bias=bqk_col)
        nc.vector.tensor_scalar(dst, zs, gam_c, bet_c, OP.mult, OP.add)

    # ---- phases A+B in one pool scope: kv rows -> kT, v; q rows -> qT, GT
    with (
        tc.tile_pool(name="xpa", bufs=8) as xp,
        tc.tile_pool(name="stpa", bufs=3) as stp,
        tc.tile_pool(name="rspa", bufs=2) as rp,
        tc.tile_pool(name="npa", bufs=3) as npool,
        tc.tile_pool(name="nTa", bufs=2) as nTp,
        tc.tile_pool(name="zpa", bufs=2) as zp,
        tc.tile_pool(name="psTra", bufs=2, space="PSUM") as psTrp,
        tc.tile_pool(name="psZa", bufs=2, space="PSUM") as psZp,
        tc.tile_pool(name="psPa", bufs=2, space="PSUM") as psPp,
    ):
        for gi in range(12):
            is_kv = gi < 8
            idx0 = gi * 4
            xts = [load_stats((xp, stp), idx0 + t) for t in range(4)]
            group_rsqrt(rp, idx0)
            nT = nTp.tile([128, 4, 512], BF16, tag="nT")
            for t in range(4):
                ln_transpose(npool, psTrp, xts[t], idx0 + t, nT, t)
            if is_kv:
                g = gi
                z_proj(psZp, zp, nT, kt_sb[:, g * 512 : (g + 1) * 512],
                       gam1_c, bet1_c)
                for t in range(4):
                    s_idx = g * 4 + t
                    psp = psPp.tile([128, HID], F32, tag="psp")
                    for nh in range(2):
                        for c in range(4):
                            nc.tensor.matmul(
                                psp[:, nh * 512 : (nh + 1) * 512],
                                nT[:, c, t * 128 : (t + 1) * 128],
                                whr[:, c, nh * 512 : (nh + 1) * 512],
                                start=(c == 0), stop=False,
                            )
                        nc.tensor.matmul(
                            psp[:, nh * 512 : (nh + 1) * 512],
                            ones_row,
                            bh_row[0:1, nh * 512 : (nh + 1) * 512],
                            start=False, stop=True,
                        )
                    nc.scalar.activation(
                        out=v_sb[:, s_idx, :], in_=psp, func=AF.Silu
                    )
            else:
                g = gi - 8
                z_proj(psZp, zp, nT, qt_sb[:, g * 512 : (g + 1) * 512],
                       gam0_c, bet0_c)
                for oc in range(8):
                    psg = psPp.tile([128, HID], F32, tag="psp")
                    for c in range(4):
                        nc.tensor.matmul(
                            psg[:, 0:512],
                            whr[:, c, HID + oc * 128 : HID + (oc + 1) * 128],
                            nT[:, c, :],
                            start=(c == 0), stop=(c == 3),
                        )
                    nc.scalar.activation(
                        out=gt_sb[:, oc, g * 512 : (g + 1) * 512],
                        in_=psg[:, 0:512],
                        func=AF.Silu, bias=bhg_t[:, oc : oc + 1],
                    )

    dbg = drams.get("dbg") or {}
    if dbg:
        nc.sync.dma_start(out=dbg["qt_early"][:], in_=qt_sb[:])
        nc.sync.dma_start(out=dbg["gt_early"][:], in_=gt_sb[:])

    # ---- phase C: sim -> relu^2 -> A@v (VT layout) -> out ----
    with (
        tc.tile_pool(name="atp", bufs=7) as atp,
        tc.tile_pool(name="rtp", bufs=3) as rtp,
        tc.tile_pool(name="vgp", bufs=2) as vgp,
        tc.tile_pool(name="osp", bufs=2) as osp,
        tc.tile_pool(name="psSim", bufs=3, space="PSUM") as pssP,
        tc.tile_pool(name="psVT", bufs=1, space="PSUM") as psVTp,
        tc.tile_pool(name="psO", bufs=1, space="PSUM") as psOp,
    ):
        def emit_out_proj(vgt, ibp):
            for ic in range(IB // 128):
                pso = psOp.tile([128, OUT], F32, tag="pso")
                for oc in range(8):
                    nc.tensor.matmul(
                        pso, vgt[:, oc, ic * 128 : (ic + 1) * 128],
                        wo_bf[:, oc, :],
                        start=(oc == 0), stop=(oc == 7),
                    )
                osb = osp.tile([128, OUT], F32, tag="osb")
                nc.vector.tensor_add(out=osb, in0=pso, in1=bo_bc)
                r0 = ibp * IB + ic * 128
                nc.sync.dma_start(out=out_d[r0 : r0 + 128, :], in_=osb)

        prev_epi = None
        for ib in range(NIB):
            # 4 oc-pair psum tiles (PSUM is bank-granular) so each pair's
            # accumulation group closes (and its vgt mul can run) while
            # later pairs still accumulate
            psvts = []
            for p in range(4):
                pvt = psVTp.tile([128, 2, IB], F32, tag=f"psvt{p}")
                psvts.append(pvt)
            ats = {}

            def emit_av(j):
                for oc in range(8):
                    # HW: start=True zeroes the WHOLE psum bank. The two oc
                    # slices of a pair share one bank, so only slice 0 may
                    # start; its bank-zero also initializes slice 1.
                    nc.tensor.matmul(
                        psvts[oc // 2][:, oc % 2, :],
                        v_sb[:, j, oc * 128 : (oc + 1) * 128],
                        ats[j],
                        start=(j == 0 and oc % 2 == 0),
                        stop=(j == NKV - 1),
                        skip_group_check=(oc % 2 == 1),
                    )

            for jt in range(NKV):
                pss = pssP.tile([128, IB], F32, tag="pss")
                nc.tensor.matmul(
                    pss,
                    kt_sb[:, jt * 128 : (jt + 1) * 128],
                    qt_sb[:, ib * IB : (ib + 1) * IB],
                    start=True, stop=True,
                )
                rt = rtp.tile([128, IB], F32, tag="rt")
                nc.scalar.activation(out=rt, in_=pss, func=AF.Relu)
                at_t = atp.tile([128, IB], BF16, tag="at")
                nc.vector.tensor_mul(out=at_t, in0=rt, in1=rt)
                ats[jt] = at_t
                if jt >= START_LAG and (jt - START_LAG) < NKV - 4:
                    emit_av(jt - START_LAG)
                if jt == 8 and prev_epi is not None:
                    # previous block's tiny output projection, emitted here
                    # so PE stays on sim/A@v while DVE finishes its vgt
                    emit_out_proj(*prev_epi)
                    prev_epi = None
            # last 4 j-tiles oc-major: psvt pair p finishes (and its vgt
            # can start) while later pairs still accumulate
            vgt = vgp.tile([128, 8, IB], BF16, tag="vgt")
            for oc in range(8):
                for j in range(NKV - 4, NKV):
                    nc.tensor.matmul(
                        psvts[oc // 2][:, oc % 2, :],
                        v_sb[:, j, oc * 128 : (oc + 1) * 128],
                        ats[j],
                        start=False, stop=(j == NKV - 1),
                    )
            for p in range(4):
                nc.vector.tensor_mul(
                    out=vgt[:, 2 * p : 2 * p + 2, :], in0=psvts[p],
                    in1=gt_sb[:, 2 * p : 2 * p + 2, ib * IB : (ib + 1) * IB],
                )
            prev_epi = (vgt, ib)
        emit_out_proj(*prev_epi)

    if dbg:
        nc.sync.dma_start(out=dbg["kt"][:], in_=kt_sb[:])
        nc.sync.dma_start(out=dbg["qt"][:], in_=qt_sb[:])
        nc.sync.dma_start(out=dbg["v"][:], in_=v_sb[:])
        nc.sync.dma_start(out=dbg["gt"][:], in_=gt_sb[:])
        nc.sync.dma_start(out=dbg["mv"][:], in_=mvall[:])
        nc.sync.dma_start(out=dbg["rstd2"][:], in_=rstd2_all[:])
        nc.sync.dma_start(out=dbg["nmr"][:], in_=nmr_all[:])


def _get_nc():
    global _nc_cache
    if _nc_cache is None:
        _nc_cache = _build()
    return _nc_cache


def _prep_in_maps(inputs):
    return _prep(**inputs)


def _prep(x, ln_g, ln_b, Wh, bh, Wqk, bqk, gamma, beta, Wo, bo):
    x = np.asarray(x, dtype=np.float32)
    f = lambda a: np.ascontiguousarray(np.asarray(a, dtype=np.float32))
    ln_g = np.asarray(ln_g, np.float64)
    ln_b = np.asarray(ln_b, np.float64)
    Whf = np.asarray(Wh, np.float64) * ln_g[:, None]
    bhf = np.asarray(bh, np.float64) + ln_b @ np.asarray(Wh, np.float64)
    Wqkf = np.asarray(Wqk, np.float64) * ln_g[:, None]
    bqkf = np.asarray(bqk, np.float64) + ln_b @ np.asarray(Wqk, np.float64)
    import ml_dtypes

    bf = lambda a: np.ascontiguousarray(
        np.asarray(a, dtype=np.float32).astype(ml_dtypes.bfloat16)
    )
    shared = {
        "wh": bf(Whf),
        "bh": bf(bhf).reshape(1, H2),
        "bhg": f(bhf[HID:]).reshape(8, 128).T,
        "wqk": bf(Wqkf),
        "bqk": f(bqkf).reshape(QK, 1),
        "gam0": f(gamma[0] / float(S)).reshape(QK, 1),
        "bet0": f(beta[0] / float(S)).reshape(QK, 1),
        "gam1": f(gamma[1]).reshape(QK, 1),
        "bet1": f(beta[1]).reshape(QK, 1),
        "wo": f(Wo).reshape(8, 128, OUT).transpose(1, 0, 2),
        "bo": f(bo).reshape(1, OUT),
    }
    shared = {k: np.ascontiguousarray(v) for k, v in shared.items()}
    in_maps = []
    for c in range(NCORES):
        b, h = c // 2, c % 2
        m = dict(shared)
        m["xkv"] = np.ascontiguousarray(x[b])
        m["xq"] = np.ascontiguousarray(x[b, h * SH : (h + 1) * SH])
        in_maps.append(m)
    return in_maps


def kernel(x, ln_g, ln_b, Wh, bh, Wqk, bqk, gamma, beta, Wo, bo):
    from concourse.bass_utils import run_bass_kernel_spmd

    nc = _get_nc()
    in_maps = _prep(x, ln_g, ln_b, Wh, bh, Wqk, bqk, gamma, beta, Wo, bo)
    res = run_bass_kernel_spmd(nc, in_maps, core_ids=list(range(NCORES)))
    out = np.empty((4, S, OUT), dtype=np.float32)
    for c in range(NCORES):
        b, h = c // 2, c % 2
        out[b, h * SH : (h + 1) * SH] = res.results[c]["out"]
    return out


# revision 8
# speedup vs baseline: 1.0553x; 1.0553x over previous
"""GAU (gated attention unit) Bass kernel for Trainium2, 8 NeuronCores — v2.

Sharding: 8 cores = 4 batches x 2 sequence halves. Each core computes the
full k/v for its batch and the attention output rows for its half.

Changes vs baseline:
  - LN rstd via fast-inverse-sqrt (bit trick + 2 Newton steps) on DVE:
    no Sqrt on Act, so the single silu_and_others table (silu/identity/
    relu/square) stays loaded forever -> 1 act-table load vs 54.
  - Projections (v/Z/gate) run in bf16 (weights shipped bf16);
    f32r transposes feed a bf16 nT via the PSUM round-trip copy.
  - Gate projected directly in transposed GT[o,i] layout (bias per
    partition, no ones-matmul; no HBM gate spill).
  - Phase C: per-jt software pipeline sim -> relu (Act) -> square (DVE)
    -> A@v accumulation in VT[o,i] layout; output projection consumes
    VT directly (no PE transposes in phase C), and each block's output
    projection is emitted inside the next block's jt loop so PE never
    drains on the DVE epilogue. The last 4 j-tiles accumulate oc-major
    so the next block's A@v never waits on the vgt epilogue chain.
    A@v psum pairs share banks: only slice 0 of each pair uses
    start=True (HW zeroes the whole bank on start).
  - Weight staging hoisted out of the rep loop.
"""
import sys

sys.path.insert(0, "/opt/trn_rl_repo")

import numpy as np

import concourse.bass as bass
import concourse.mybir as mybir
from concourse import bacc
from concourse.masks import make_identity
from concourse.tile import TileContext

F32 = mybir.dt.float32
F32R = mybir.dt.float32r
BF16 = mybir.dt.bfloat16
I32 = mybir.dt.int32
AF = mybir.ActivationFunctionType
OP = mybir.AluOpType

S = 4096          # full sequence
SH = 2048         # per-core q rows
D = 512           # model dim
HID = 1024        # v / gate width
H2 = 2048         # 2*HID
QK = 128
OUT = 8
NKV = S // 128    # 32 kv seq tiles
NQT = SH // 128   # 16 q seq tiles
NT = NKV + NQT    # 48 stats tiles
IB = 256          # phase-C i-block rows
NIB = SH // IB    # 8 blocks
NCORES = 8
START_LAG = 4     # sim -> A@v software pipeline depth

_nc_cache = None


def _build(nreps=1, debug=False):
    nc = bacc.Bacc()

    xkv = nc.dram_tensor("xkv", [S, D], F32, kind="ExternalInput")
    xq = nc.dram_tensor("xq", [SH, D], F32, kind="ExternalInput")
    # projection weights shipped bf16 (converted host-side), DMA'd straight
    # into bf16 SBUF tiles with no staging copy
    wh = nc.dram_tensor("wh", [D, H2], BF16, kind="ExternalInput")
    bh = nc.dram_tensor("bh", [1, H2], BF16, kind="ExternalInput")
    wqk = nc.dram_tensor("wqk", [D, QK], BF16, kind="ExternalInput")
    bqk = nc.dram_tensor("bqk", [QK, 1], F32, kind="ExternalInput")
    gam0 = nc.dram_tensor("gam0", [QK, 1], F32, kind="ExternalInput")
    bet0 = nc.dram_tensor("bet0", [QK, 1], F32, kind="ExternalInput")
    gam1 = nc.dram_tensor("gam1", [QK, 1], F32, kind="ExternalInput")
    bet1 = nc.dram_tensor("bet1", [QK, 1], F32, kind="ExternalInput")
    wo = nc.dram_tensor("wo", [128, 8, OUT], F32, kind="ExternalInput")
    bo = nc.dram_tensor("bo", [1, OUT], F32, kind="ExternalInput")
    bhg = nc.dram_tensor("bhg", [128, 8], F32, kind="ExternalInput")
    out_d = nc.dram_tensor("out", [SH, OUT], F32, kind="ExternalOutput")
    dbg = {}
    if debug:
        dbg["kt"] = nc.dram_tensor("dbg_kt", [128, S], F32R, kind="ExternalOutput")
        dbg["qt"] = nc.dram_tensor("dbg_qt", [128, SH], F32R, kind="ExternalOutput")
        dbg["v"] = nc.dram_tensor("dbg_v", [128, NKV * HID], BF16, kind="ExternalOutput")
        dbg["gt"] = nc.dram_tensor("dbg_gt", [128, 8 * SH], F32R, kind="ExternalOutput")
        dbg["mv"] = nc.dram_tensor("dbg_mv", [128, NT * 2], F32, kind="ExternalOutput")
        dbg["rstd2"] = nc.dram_tensor("dbg_rstd2", [128, NT], F32, kind="ExternalOutput")
        dbg["nmr"] = nc.dram_tensor("dbg_nmr", [128, NT], F32, kind="ExternalOutput")
        dbg["qt_early"] = nc.dram_tensor("dbg_qt_early", [128, SH], F32R, kind="ExternalOutput")
        dbg["gt_early"] = nc.dram_tensor("dbg_gt_early", [128, 8 * SH], F32R, kind="ExternalOutput")

    with TileContext(nc) as tc:
        with (
            tc.tile_pool(name="persist", bufs=1) as pers,
            tc.tile_pool(name="vpool", bufs=1) as vpool,
        ):
            # ---- persistent constants ----
            identity = pers.tile([128, 128], F32, tag="ident")
            make_identity(nc, identity)
            ident_r = pers.tile([128, 128], F32R, tag="identr")
            nc.vector.tensor_copy(out=ident_r, in_=identity)

            bqk_col = pers.tile([128, 1], F32, tag="bqk")
            nc.sync.dma_start(out=bqk_col, in_=bqk[:])
            gam0_c = pers.tile([128, 1], F32, tag="g0")
            nc.sync.dma_start(out=gam0_c, in_=gam0[:])
            bet0_c = pers.tile([128, 1], F32, tag="be0")
            nc.sync.dma_start(out=bet0_c, in_=bet0[:])
            gam1_c = pers.tile([128, 1], F32, tag="g1")
            nc.sync.dma_start(out=gam1_c, in_=gam1[:])
            bet1_c = pers.tile([128, 1], F32, tag="be1")
            nc.sync.dma_start(out=bet1_c, in_=bet1[:])
            wo_t = pers.tile([128, 8, OUT], F32, tag="wo")
            nc.sync.dma_start(out=wo_t, in_=wo[:])
            wo_bf = pers.tile([128, 8, OUT], BF16, tag="wobf")
            nc.vector.tensor_copy(out=wo_bf, in_=wo_t)
            bo_bc = pers.tile([128, OUT], F32, tag="bo")
            nc.sync.dma_start(out=bo_bc, in_=bo[:].to_broadcast([128, OUT]))
            bhg_t = pers.tile([128, 8], F32, tag="bhg")
            nc.sync.dma_start(out=bhg_t, in_=bhg[:])

            # ---- staged weights (outside the rep loop, direct bf16 DMA) ----
            whr = pers.tile([128, 4, H2], BF16, tag="whr")
            for c in range(4):
                nc.sync.dma_start(
                    out=whr[:, c, :], in_=wh[c * 128 : (c + 1) * 128, :]
                )
            wqkr = pers.tile([128, 4, QK], BF16, tag="wqkr")
            for c in range(4):
                nc.sync.dma_start(
                    out=wqkr[:, c, :], in_=wqk[c * 128 : (c + 1) * 128, :]
                )
            ones_row = pers.tile([1, 128], BF16, tag="ones")
            nc.vector.memset(ones_row, 1.0)
            bh_row = pers.tile([1, HID], BF16, tag="bhr")
            nc.sync.dma_start(out=bh_row, in_=bh[0:1, 0:HID])

            # ---- persistent activations ----
            v_sb = vpool.tile([128, NKV, HID], BF16, tag="v")
            kt_sb = pers.tile([128, S], F32R, tag="kt")
            qt_sb = pers.tile([128, SH], F32R, tag="qt")
            gt_sb = pers.tile([128, 8, SH], BF16, tag="gt")
            mvall = pers.tile([128, NT, 2], F32, tag="mvall")
            rstd2_all = pers.tile([128, NT], F32, tag="rstd2")
            nmr_all = pers.tile([128, NT], F32, tag="nmr")

            sbufs = dict(
                ident_r=ident_r, ones_row=ones_row,
                bqk_col=bqk_col, gam0_c=gam0_c, bet0_c=bet0_c,
                gam1_c=gam1_c, bet1_c=bet1_c, wo_bf=wo_bf, bo_bc=bo_bc,
                bhg_t=bhg_t, whr=whr, wqkr=wqkr, bh_row=bh_row,
                v_sb=v_sb, kt_sb=kt_sb, qt_sb=qt_sb, gt_sb=gt_sb,
                mvall=mvall, rstd2_all=rstd2_all, nmr_all=nmr_all,
            )
            drams = dict(xkv=xkv, xq=xq, out_d=out_d, dbg=dbg)

            import contextlib

            rep_ctx = (
                tc.For_i(0, nreps, 1) if nreps > 1 else contextlib.nullcontext()
            )
            with rep_ctx:
                _emit_body(nc, tc, drams, sbufs)

    nc.compile()
    return nc


def _emit_body(nc, tc, drams, sb):
    xkv, xq, out_d = drams["xkv"], drams["xq"], drams["out_d"]
    ident_r = sb["ident_r"]
    ones_row = sb["ones_row"]
    bqk_col = sb["bqk_col"]
    gam0_c, bet0_c = sb["gam0_c"], sb["bet0_c"]
    gam1_c, bet1_c = sb["gam1_c"], sb["bet1_c"]
    wo_bf, bo_bc, bhg_t = sb["wo_bf"], sb["bo_bc"], sb["bhg_t"]
    whr, wqkr, bh_row = sb["whr"], sb["wqkr"], sb["bh_row"]
    v_sb, kt_sb, qt_sb, gt_sb = sb["v_sb"], sb["kt_sb"], sb["qt_sb"], sb["gt_sb"]
    mvall = sb["mvall"]
    rstd2_all, nmr_all = sb["rstd2_all"], sb["nmr_all"]

    def tile_src(idx):
        if idx < NKV:
            return xkv, idx * 128
        return xq, (idx - NKV) * 128

    def group_rsqrt(rp, idx0, n=4):
        """rstd2_all[:, idx0:idx0+n] = rsqrt(var + eps) via bit trick +
        2 Newton steps; nmr_all = -mean * rstd2. All on DVE."""
        ve = rp.tile([128, n], F32, tag="ve")
        nc.vector.tensor_scalar_add(out=ve, in0=mvall[:, idx0 : idx0 + n, 1],
                                    scalar1=1e-5)
        ib1 = rp.tile([128, n], I32, tag="ib1")
        nc.vector.tensor_scalar(
            out=ib1, in0=ve.bitcast(I32), scalar1=1, scalar2=None,
            op0=OP.arith_shift_right,
        )
        ib2 = rp.tile([128, n], I32, tag="ib2")
        # C - a == -(a - C); bitwise+arith can't mix in one tensor_scalar
        nc.vector.tensor_scalar(
            out=ib2, in0=ib1, scalar1=0x5F3759DF, scalar2=-1,
            op0=OP.subtract, op1=OP.mult,
        )
        y = ib2.bitcast(F32)
        for it in range(2):
            yy = rp.tile([128, n], F32, tag=f"yy{it}")
            nc.vector.tensor_mul(out=yy, in0=y, in1=y)
            xyy = rp.tile([128, n], F32, tag=f"xyy{it}")
            nc.vector.tensor_mul(out=xyy, in0=yy, in1=ve)
            h = rp.tile([128, n], F32, tag=f"h{it}")
            nc.vector.tensor_scalar(
                out=h, in0=xyy, scalar1=-0.5, scalar2=1.5,
                op0=OP.mult, op1=OP.add,
            )
            if it == 0:
                ynext = rp.tile([128, n], F32, tag=f"y{it}")
            else:
                ynext = rstd2_all[:, idx0 : idx0 + n]
            nc.vector.tensor_mul(out=ynext, in0=y, in1=h)
            y = ynext
        nc.vector.scalar_tensor_tensor(
            out=nmr_all[:, idx0 : idx0 + n], in0=mvall[:, idx0 : idx0 + n, 0],
            scalar=-1.0, in1=rstd2_all[:, idx0 : idx0 + n],
            op0=OP.mult, op1=OP.mult,
        )

    def load_stats(pools, idx):
        """DMA one x tile and push bn stats into mvall[:, idx]."""
        xp, stp = pools
        src, row0 = tile_src(idx)
        xt = xp.tile([128, D], F32, tag="xa")
        nc.sync.dma_start(out=xt, in_=src[row0 : row0 + 128, :])
        st = stp.tile([128, 6], F32, tag="bn")
        nc.vector.bn_stats(out=st, in_=xt)
        nc.vector.bn_aggr(out=mvall[:, idx, :], in_=st)
        return xt

    def ln_transpose(npool, psTrp, xt, idx, nT, t):
        nsc = npool.tile([128, D], F32R, tag="nsc")
        # LN apply on Act: x*rstd + (-mean*rstd)
        nc.scalar.activation(
            out=nsc, in_=xt, func=AF.Identity,
            bias=nmr_all[:, idx : idx + 1], scale=rstd2_all[:, idx : idx + 1],
        )
        ptr = psTrp.tile([128, 4, 128], F32R, tag="ptr")
        for c in range(4):
            nc.tensor.transpose(
                ptr[:, c, :], nsc[:, c * 128 : (c + 1) * 128], ident_r
            )
        nc.vector.tensor_copy(out=nT[:, :, t * 128 : (t + 1) * 128], in_=ptr)

    def z_proj(psZp, zp, nT, dst, gam_c, bet_c):
        psz = psZp.tile([128, 512], F32, tag="psz")
        for c in range(4):
            nc.tensor.matmul(
                psz, wqkr[:, c, :], nT[:, c, :], start=(c == 0), stop=(c == 3)
            )
        zs = zp.tile([128, 512], F32, tag="zs")
        nc.scalar.activation(out=zs, in_=psz, func=AF.Silu, bias=bqk_col)
        nc.vector.tensor_scalar(dst, zs, gam_c, bet_c, OP.mult, OP.add)

    # ---- phases A+B in one pool scope: kv rows -> kT, v; q rows -> qT, GT
    with (
        tc.tile_pool(name="xpa", bufs=8) as xp,
        tc.tile_pool(name="stpa", bufs=3) as stp,
        tc.tile_pool(name="rspa", bufs=2) as rp,
        tc.tile_pool(name="npa", bufs=3) as npool,
        tc.tile_pool(name="nTa", bufs=2) as nTp,
        tc.tile_pool(name="zpa", bufs=2) as zp,
        tc.tile_pool(name="psTra", bufs=2, space="PSUM") as psTrp,
        tc.tile_pool(name="psZa", bufs=2, space="PSUM") as psZp,
        tc.tile_pool(name="psPa", bufs=2, space="PSUM") as psPp,
    ):
        for gi in range(12):
            is_kv = gi < 8
            idx0 = gi * 4
            xts = [load_stats((xp, stp), idx0 + t) for t in range(4)]
            group_rsqrt(rp, idx0)
            nT = nTp.tile([128, 4, 512], BF16, tag="nT")
            for t in range(4):
                ln_transpose(npool, psTrp, xts[t], idx0 + t, nT, t)
            if is_kv:
                g = gi
                z_proj(psZp, zp, nT, kt_sb[:, g * 512 : (g + 1) * 512],
                       gam1_c, bet1_c)
                for t in range(4):
                    s_idx = g * 4 + t
                    psp = psPp.tile([128, HID], F32, tag="psp")
                    for nh in range(2):
                        for c in range(4):
                            nc.tensor.matmul(
                                psp[:, nh * 512 : (nh + 1) * 512],
                                nT[:, c, t * 128 : (t + 1) * 128],
                                whr[:, c, nh * 512 : (nh + 1) * 512],
                                start=(c == 0), stop=False,
                            )
                        nc.tensor.matmul(
                            psp[:, nh * 512 : (nh + 1) * 512],
                            ones_row,
                            bh_row[0:1, nh * 512 : (nh + 1) * 512],
                            start=False, stop=True,
                        )
                    nc.scalar.activation(
                        out=v_sb[:, s_idx, :], in_=psp, func=AF.Silu
                    )
            else:
                g = gi - 8
                z_proj(psZp, zp, nT, qt_sb[:, g * 512 : (g + 1) * 512],
                       gam0_c, bet0_c)
                for oc in range(8):
                    psg = psPp.tile([128, HID], F32, tag="psp")
                    for c in range(4):
                        nc.tensor.matmul(
                            psg[:, 0:512],
                            whr[:, c, HID + oc * 128 : HID + (oc + 1) * 128],
                            nT[:, c, :],
                            start=(c == 0), stop=(c == 3),
                        )
                    nc.scalar.activation(
                        out=gt_sb[:, oc, g * 512 : (g + 1) * 512],
                        in_=psg[:, 0:512],
                        func=AF.Silu, bias=bhg_t[:, oc : oc + 1],
                    )

    dbg = drams.get("dbg") or {}
    if dbg:
        nc.sync.dma_start(out=dbg["qt_early"][:], in_=qt_sb[:])
        nc.sync.dma_start(out=dbg["gt_early"][:], in_=gt_sb[:])

    # ---- phase C: sim -> relu^2 -> A@v (VT layout) -> out ----
    # sim/relu/square run at 512 width covering TWO 256-row blocks per
    # super-block (halves their instruction count); all 32 at tiles stay
    # resident so the second block is a pure A@v sweep.
    with (
        tc.tile_pool(name="atp", bufs=34) as atp,
        tc.tile_pool(name="rtp", bufs=3) as rtp,
        tc.tile_pool(name="vgp", bufs=2) as vgp,
        tc.tile_pool(name="osp", bufs=2) as osp,
        tc.tile_pool(name="psSim", bufs=3, space="PSUM") as pssP,
        tc.tile_pool(name="psVT", bufs=1, space="PSUM") as psVTp,
        tc.tile_pool(name="psO", bufs=1, space="PSUM") as psOp,
    ):
        def emit_out_proj(vgt, ibp):
            for ic in range(IB // 128):
                pso = psOp.tile([128, OUT], F32, tag="pso")
                for oc in range(8):
                    nc.tensor.matmul(
                        pso, vgt[:, oc, ic * 128 : (ic + 1) * 128],
                        wo_bf[:, oc, :],
                        start=(oc == 0), stop=(oc == 7),
                    )
                osb = osp.tile([128, OUT], F32, tag="osb")
                nc.vector.tensor_add(out=osb, in0=pso, in1=bo_bc)
                r0 = ibp * IB + ic * 128
                nc.sync.dma_start(out=out_d[r0 : r0 + 128, :], in_=osb)

        prev_epi = None
        for sb in range(NIB // 2):
            ats = {}

            def alloc_psvts():
                # 4 oc-pair psum tiles (PSUM is bank-granular) so each
                # pair's accumulation group closes (and its vgt mul can
                # run) while later pairs still accumulate
                ps = []
                for p in range(4):
                    pvt = psVTp.tile([128, 2, IB], F32, tag=f"psvt{p}")
                    ps.append(pvt)
                return ps

            def emit_av(psvts, j, half, j_first):
                c0 = half * IB
                for oc in range(8):
                    # HW: start=True zeroes the WHOLE psum bank. The two oc
                    # slices of a pair share one bank, so only slice 0 may
                    # start; its bank-zero also initializes slice 1.
                    nc.tensor.matmul(
                        psvts[oc // 2][:, oc % 2, :],
                        v_sb[:, j, oc * 128 : (oc + 1) * 128],
                        ats[j][:, c0 : c0 + IB],
                        start=(j == j_first and oc % 2 == 0),
                        stop=False,
                        skip_group_check=(oc % 2 == 1),
                    )

            def emit_tail_and_vgt(psvts, half, ibp):
                # last 4 j-tiles oc-major: psvt pair p finishes (and its
                # vgt can start) while later pairs still accumulate
                c0 = half * IB
                vgt = vgp.tile([128, 8, IB], BF16, tag="vgt")
                for oc in range(8):
                    for j in range(NKV - 4, NKV):
                        nc.tensor.matmul(
                            psvts[oc // 2][:, oc % 2, :],
                            v_sb[:, j, oc * 128 : (oc + 1) * 128],
                            ats[j][:, c0 : c0 + IB],
                            start=False, stop=(j == NKV - 1),
                            skip_group_check=True,
                        )
                for p in range(4):
                    nc.vector.tensor_mul(
                        out=vgt[:, 2 * p : 2 * p + 2, :], in0=psvts[p],
                        in1=gt_sb[:, 2 * p : 2 * p + 2,
                                  ibp * IB : (ibp + 1) * IB],
                    )
                return vgt

            # block A (ib = 2*sb): sims at 512 width, A@v on cols 0:IB
            psvtsA = alloc_psvts()
            for jt in range(NKV):
                pss = pssP.tile([128, 2 * IB], F32, tag="pss")
                nc.tensor.matmul(
                    pss,
                    kt_sb[:, jt * 128 : (jt + 1) * 128],
                    qt_sb[:, sb * 2 * IB : (sb + 1) * 2 * IB],
                    start=True, stop=True,
                )
                rt = rtp.tile([128, 2 * IB], F32, tag="rt")
                nc.scalar.activation(out=rt, in_=pss, func=AF.Relu)
                at_t = atp.tile([128, 2 * IB], BF16, tag="at")
                nc.vector.tensor_mul(out=at_t, in0=rt, in1=rt)
                ats[jt] = at_t
                if jt >= START_LAG and (jt - START_LAG) < NKV - 4:
                    emit_av(psvtsA, jt - START_LAG, 0, 0)
                if jt == 8 and prev_epi is not None:
                    # previous block's tiny output projection, emitted here
                    # so PE stays on sim/A@v while DVE finishes its vgt
                    emit_out_proj(*prev_epi)
                    prev_epi = None
            vgtA = emit_tail_and_vgt(psvtsA, 0, 2 * sb)
            # block B (ib = 2*sb+1): pure A@v sweep over resident ats
            psvtsB = alloc_psvts()
            for jt in range(NKV - 4):
                emit_av(psvtsB, jt, 1, 0)
                if jt == 8:
                    emit_out_proj(vgtA, 2 * sb)
            vgtB = emit_tail_and_vgt(psvtsB, 1, 2 * sb + 1)
            prev_epi = (vgtB, 2 * sb + 1)
        emit_out_proj(*prev_epi)

    if dbg:
        nc.sync.dma_start(out=dbg["kt"][:], in_=kt_sb[:])
        nc.sync.dma_start(out=dbg["qt"][:], in_=qt_sb[:])
        nc.sync.dma_start(out=dbg["v"][:], in_=v_sb[:])
        nc.sync.dma_start(out=dbg["gt"][:], in_=gt_sb[:])
        nc.sync.dma_start(out=dbg["mv"][:], in_=mvall[:])
        nc.sync.dma_start(out=dbg["rstd2"][:], in_=rstd2_all[:])
        nc.sync.dma_start(out=dbg["nmr"][:], in_=nmr_all[:])


def _get_nc():
    global _nc_cache
    if _nc_cache is None:
        _nc_cache = _build()
    return _nc_cache


def _prep_in_maps(inputs):
    return _prep(**inputs)


def _prep(x, ln_g, ln_b, Wh, bh, Wqk, bqk, gamma, beta, Wo, bo):
    x = np.asarray(x, dtype=np.float32)
    f = lambda a: np.ascontiguousarray(np.asarray(a, dtype=np.float32))
    ln_g = np.asarray(ln_g, np.float64)
    ln_b = np.asarray(ln_b, np.float64)
    Whf = np.asarray(Wh, np.float64) * ln_g[:, None]
    bhf = np.asarray(bh, np.float64) + ln_b @ np.asarray(Wh, np.float64)
    Wqkf = np.asarray(Wqk, np.float64) * ln_g[:, None]
    bqkf = np.asarray(bqk, np.float64) + ln_b @ np.asarray(Wqk, np.float64)
    import ml_dtypes

    bf = lambda a: np.ascontiguousarray(
        np.asarray(a, dtype=np.float32).astype(ml_dtypes.bfloat16)
    )
    shared = {
        "wh": bf(Whf),
        "bh": bf(bhf).reshape(1, H2),
        "bhg": f(bhf[HID:]).reshape(8, 128).T,
        "wqk": bf(Wqkf),
        "bqk": f(bqkf).reshape(QK, 1),
        "gam0": f(gamma[0] / float(S)).reshape(QK, 1),
        "bet0": f(beta[0] / float(S)).reshape(QK, 1),
        "gam1": f(gamma[1]).reshape(QK, 1),
        "bet1": f(beta[1]).reshape(QK, 1),
        "wo": f(Wo).reshape(8, 128, OUT).transpose(1, 0, 2),
        "bo": f(bo).reshape(1, OUT),
    }
    shared = {k: np.ascontiguousarray(v) for k, v in shared.items()}
    in_maps = []
    for c in range(NCORES):
        b, h = c // 2, c % 2
        m = dict(shared)
        m["xkv"] = np.ascontiguousarray(x[b])
        m["xq"] = np.ascontiguousarray(x[b, h * SH : (h + 1) * SH])
        in_maps.append(m)
    return in_maps


def kernel(x, ln_g, ln_b, Wh, bh, Wqk, bqk, gamma, beta, Wo, bo):
    from concourse.bass_utils import run_bass_kernel_spmd

    nc = _get_nc()
    in_maps = _prep(x, ln_g, ln_b, Wh, bh, Wqk, bqk, gamma, beta, Wo, bo)
    res = run_bass_kernel_spmd(nc, in_maps, core_ids=list(range(NCORES)))
    out = np.empty((4, S, OUT), dtype=np.float32)
    for c in range(NCORES):
        b, h = c // 2, c % 2
        out[b, h * SH : (h + 1) * SH] = res.results[c]["out"]
    return out
